# revision 18
# baseline (speedup 1.0000x reference)
"""AI4DEM coupling_backward on 8 TRN2 NeuronCores.

Math: for each of 9 taps (sy,sx) in {-1,0,1}^2, the reference computes
  w = N(roll(xp)-x) * N(roll(yp)-y) * mask * VP   with N(d) = max(0, 1-|d|)
  out = [sum w, sum w*roll(vx), sum w*roll(vy), sum w*roll(Fx), sum w*roll(Fy)]
Per-source factorization with ux = xp - x, uy = yp - y (computed during
host-side sharding, identical f32 op to the reference's roll(xp)-x):
  C_sx = tent(ux - sx), R_sy = tent(uy - sy)
  out_f[y,x] = mask[y,x]*VP * sum_{sy,sx} (C_sx*R_sy*v_f)[y-sy, x-sx]
Circular wrap taps vanish automatically because halo cells carry *virtual*
(unwrapped) coordinates, making |d| huge -> tent = 0.

Layout: shard 512 rows/core (+1 halo row each side). On device, partitions =
128 column strips of 34 (32 owned + 1 halo col each side); free dim =
(rows, 34). All 9 stencil shifts are then free-dim offsets. ScalarE computes
the 6 tent weights, VectorE the 45 products in bf16 (2x mode, fused 3 taps
per instruction via stacked tiles + broadcast APs), TensorE accumulates the
9 taps per field into PSUM via identity matmuls; evac multiplies by mask*VP.

This toolchain's walrus allows ONE sync-wait per instruction; the
split_multi_waits pass hoists extra waits onto same-engine NOPs.
"""

import numpy as np
import ml_dtypes

from concourse import bass, mybir
from concourse import tile as tile_mod
from concourse.bass_utils import run_bass_kernel_spmd

NY, NX = 4096, 4096
VP = 3.1415 / 6.0
NCORES = 8
ROWS = NY // NCORES          # 512 output rows per core
P = 128                      # partitions = column strips
OW = NX // P                 # 32 output cols per strip
SW = OW + 2                  # 34 strip cols incl halo
SRC_ROWS = ROWS + 2          # 514 source rows per core

RCH = 64                     # output rows per chunk
SR = RCH + 2                 # source rows per chunk window
NCHUNK = ROWS // RCH
WIN = 16                     # psum window rows (16*32 = 512 = one psum bank)
NWIN = RCH // WIN

F32 = mybir.dt.float32
PROD_DT = mybir.dt.bfloat16  # dtype for weights/products
PROD_NP = ml_dtypes.bfloat16

_CACHE = {}


def split_multi_waits(nc):
    """Walrus allows one sync-wait per instruction; hoist extras onto NOPs."""
    for f in nc.m.functions:
        for bb in f.blocks:
            new = []
            for inst in bb.instructions:
                si = inst.sync_info
                if si is not None and si.on_wait and len(si.on_wait) > 1:
                    waits = list(si.on_wait)
                    for w in waits[:-1]:
                        nop = mybir.InstNoOp(name=f"W-{nc.next_id()}",
                                             ins=[], outs=[])
                        nop.engine = inst.engine
                        nop.sync_info = mybir.SyncInfo(on_wait=[w],
                                                       on_update=[])
                        new.append(nop)
                    inst.sync_info = mybir.SyncInfo(
                        on_wait=[waits[-1]], on_update=list(si.on_update))
                new.append(inst)
            bb.instructions = new


def _build_graph(repeat=1):
    nc = bass.Bass()
    halo_shape = [P, SRC_ROWS, SW]
    ux_d = nc.declare_dram_parameter("ux", halo_shape, F32, isOutput=False)
    uy_d = nc.declare_dram_parameter("uy", halo_shape, F32, isOutput=False)
    v_d = [nc.declare_dram_parameter(n, halo_shape, F32, isOutput=False)
           for n in ("vx", "vy", "fx", "fy")]
    mask_d = nc.declare_dram_parameter("mask", [P, ROWS, OW], F32, isOutput=False)
    eye_d = nc.declare_dram_parameter("eye", [P, P], PROD_DT, isOutput=False)
    out_d = nc.declare_dram_parameter("out", [5, P, ROWS, OW], F32, isOutput=True)

    # taps ordered sy-major so W9[:, 3*i + j] = R(sy_i) * C(sx_j)
    SYS = (1, 0, -1)
    SXS = (1, 0, -1)
    TAPS = [(sy, sx) for sy in SYS for sx in SXS]

    with tile_mod.TileContext(nc) as tc:
        with (
            tc.tile_pool(name="const", bufs=1) as cpool,
            tc.tile_pool(name="ldxy", bufs=2) as ldxy,
            tc.tile_pool(name="ldv", bufs=2) as ldv,
            tc.tile_pool(name="ldm", bufs=1) as ldm,
            tc.tile_pool(name="wk", bufs=1) as wkpool,
            tc.tile_pool(name="wt", bufs=1) as wtpool,
            tc.tile_pool(name="prod", bufs=3) as ppool,
            tc.tile_pool(name="outp", bufs=4) as opool,
            tc.tile_pool(name="psum", bufs=6, space="PSUM") as pspool,
        ):
            eye_t = cpool.tile([P, P], PROD_DT, tag="eye")
            nc.sync.dma_start(eye_t[:], eye_d[:])
            # per-partition-scalar bias columns (-s for s in 1,0,-1), ACT-owned
            bias_g = cpool.tile([P, 3], F32, tag="bias_g")
            for k, s in enumerate(SXS):
                nc.gpsimd.memset(bias_g[:, k:k + 1], float(-s))
            bias_t = cpool.tile([P, 3], F32, tag="bias")
            nc.scalar.copy(bias_t[:], bias_g[:])
            bias_col = {s: bias_t[:, k:k + 1] for k, s in enumerate(SXS)}

            for t in range(NCHUNK * repeat):
                t = t % NCHUNK
                r0 = t * RCH
                ux_t = ldxy.tile([P, SR, SW], PROD_DT, tag="ux")
                uy_t = ldxy.tile([P, SR, SW], PROD_DT, tag="uy")
                nc.gpsimd.dma_start(ux_t[:], ux_d[:, r0:r0 + SR, :])
                nc.gpsimd.dma_start(uy_t[:], uy_d[:, r0:r0 + SR, :])
                v_t = []
                for k, vd in enumerate(v_d):
                    vt = ldv.tile([P, 1, SR, SW], PROD_DT, tag=f"v{k}")
                    nc.gpsimd.dma_start(vt[:, 0], vd[:, r0:r0 + SR, :])
                    v_t.append(vt)
                mask_t = ldm.tile([P, RCH, OW], F32, tag="mask")
                nc.sync.dma_start(mask_t[:], mask_d[:, r0:r0 + RCH, :])
                # mc = mask * VP on ACT (keeps DVE off 2-port modes)
                mc_t = wkpool.tile([P, RCH, OW], F32, tag="mc")
                nc.scalar.mul(mc_t[:], mask_t[:], float(VP))

                # tent weights on ScalarE: tent(d) = relu(1 - |d|), stacked
                CW = wtpool.tile([P, 3, SR, SW], PROD_DT, tag="CW")
                RW = wtpool.tile([P, 3, SR, SW], PROD_DT, tag="RW")
                for k, s in enumerate(SXS):
                    ax = wkpool.tile([P, SR, SW], PROD_DT, tag="scr")
                    nc.scalar.activation(ax[:], ux_t[:],
                                         mybir.ActivationFunctionType.Abs,
                                         bias=bias_col[s])
                    nc.scalar.activation(CW[:, k], ax[:],
                                         mybir.ActivationFunctionType.Relu,
                                         bias=1.0, scale=-1.0)
                    ay = wkpool.tile([P, SR, SW], PROD_DT, tag="scr")
                    nc.scalar.activation(ay[:], uy_t[:],
                                         mybir.ActivationFunctionType.Abs,
                                         bias=bias_col[s])
                    nc.scalar.activation(RW[:, k], ay[:],
                                         mybir.ActivationFunctionType.Relu,
                                         bias=1.0, scale=-1.0)

                # 9 tap weights W9[:, 3i+j] = RW[i] * CW[j], 3 fused muls
                W9 = wtpool.tile([P, 9, SR, SW], PROD_DT, tag="W9")
                for i in range(3):
                    nc.vector.tensor_mul(
                        W9[:, 3 * i:3 * i + 3],
                        RW[:, i:i + 1].to_broadcast((P, 3, SR, SW)),
                        CW[:])

                for f in range(5):
                    ps = []
                    for w in range(NWIN):
                        ps_w = pspool.tile([P, WIN, OW], F32, tag="ps")
                        ps.append(ps_w)
                    for g in range(3):          # tap group = fixed sy
                        if f == 0:
                            src = W9[:, 3 * g:3 * g + 3]
                        else:
                            t3 = ppool.tile([P, 3, SR, SW], PROD_DT, tag="t3")
                            nc.vector.tensor_mul(
                                t3[:], W9[:, 3 * g:3 * g + 3],
                                v_t[f - 1][:].to_broadcast((P, 3, SR, SW)))
                            src = t3[:]
                        d = 1 - SYS[g]
                        for j in range(3):
                            ti = 3 * g + j
                            c0 = 1 - SXS[j]
                            for w in range(NWIN):
                                nc.tensor.matmul(
                                    ps[w][:],
                                    eye_t[:],
                                    src[:, j, w * WIN + d: w * WIN + d + WIN,
                                        c0:c0 + OW],
                                    start=(ti == 0), stop=(ti == 8),
                                )
                    for w in range(NWIN):
                        ot = opool.tile([P, WIN, OW], F32, tag="out")
                        nc.vector.tensor_mul(
                            ot[:], ps[w][:],
                            mc_t[:, w * WIN:(w + 1) * WIN, :])
                        nc.sync.dma_start(
                            out_d[f, :, r0 + w * WIN:r0 + (w + 1) * WIN, :],
                            ot[:])
    split_multi_waits(nc)
    return nc


def _strips_halo(arr2d):
    """(rows, NX) f32 -> [P, rows, SW] with 1-col circular halos."""
    rows = arr2d.shape[0]
    pad = np.empty((rows, NX + 2), np.float32)
    pad[:, 1:NX + 1] = arr2d
    pad[:, 0] = arr2d[:, NX - 1]
    pad[:, NX + 1] = arr2d[:, 0]
    st = pad.strides
    v = np.lib.stride_tricks.as_strided(
        pad, shape=(P, rows, SW), strides=(OW * st[1], st[0], st[1]))
    return np.ascontiguousarray(v)


# virtual (unwrapped) column coordinate of each strip cell: 32p - 1 + c
_XC = (OW * np.arange(P, dtype=np.float32)[:, None, None] - 1.0
       + np.arange(SW, dtype=np.float32)[None, None, :])


def build_in_maps(xp_grid, yp_grid, vx_grid, vy_grid, Fx_grid, Fy_grid, mask):
    xp = np.asarray(xp_grid, np.float32).reshape(NY, NX)
    yp = np.asarray(yp_grid, np.float32).reshape(NY, NX)
    fields = [np.asarray(a, np.float32).reshape(NY, NX)
              for a in (vx_grid, vy_grid, Fx_grid, Fy_grid)]
    mk = np.asarray(mask, np.float32).reshape(NY, NX)
    eye = np.eye(P, dtype=PROD_NP)

    in_maps = []
    for i in range(NCORES):
        r0 = i * ROWS
        ridx = np.arange(r0 - 1, r0 + ROWS + 1) % NY
        ux = _strips_halo(xp[ridx])
        ux -= _XC                      # xp - x (virtual, unwrapped)
        uy = _strips_halo(yp[ridx])
        uy -= (np.arange(r0 - 1, r0 + ROWS + 1, dtype=np.float32)
               [None, :, None])       # yp - y (virtual, unwrapped)
        m = {
            "ux": ux,
            "uy": uy,
            "mask": np.ascontiguousarray(
                mk[r0:r0 + ROWS].reshape(ROWS, P, OW).transpose(1, 0, 2)),
            "eye": eye,
        }
        for nm, f in zip(("vx", "vy", "fx", "fy"), fields):
            m[nm] = _strips_halo(f[ridx])
        in_maps.append(m)
    return in_maps


def kernel(xp_grid, yp_grid, x_grid, y_grid, vx_grid, vy_grid, Fx_grid,
           Fy_grid, mask):
    if "nc" not in _CACHE:
        _CACHE["nc"] = _build_graph()
    nc = _CACHE["nc"]
    in_maps = build_in_maps(xp_grid, yp_grid, vx_grid, vy_grid, Fx_grid,
                            Fy_grid, mask)
    res = run_bass_kernel_spmd(nc, in_maps, core_ids=list(range(NCORES)))
    out = np.empty((5, 1, 1, NY, NX), np.float32)
    for i in range(NCORES):
        o = np.asarray(res.results[i]["out"])  # [5, P, ROWS, OW]
        out[:, 0, 0, i * ROWS:(i + 1) * ROWS, :] = (
            o.transpose(0, 2, 1, 3).reshape(5, ROWS, NX))
    return out


# revision 19
# speedup vs baseline: 1.0814x; 1.0814x over previous
"""AI4DEM coupling_backward on 8 TRN2 NeuronCores.

Math: for each of 9 taps (sy,sx) in {-1,0,1}^2, the reference computes
  w = N(roll(xp)-x) * N(roll(yp)-y) * mask * VP   with N(d) = max(0, 1-|d|)
  out = [sum w, sum w*roll(vx), sum w*roll(vy), sum w*roll(Fx), sum w*roll(Fy)]
Per-source factorization with ux = xp - x, uy = yp - y (computed during
host-side sharding, identical f32 op to the reference's roll(xp)-x):
  C_sx = tent(ux - sx), R_sy = tent(uy - sy)
  out_f[y,x] = mask[y,x]*VP * sum_{sy,sx} (C_sx*R_sy*v_f)[y-sy, x-sx]
Circular wrap taps vanish automatically because halo cells carry *virtual*
(unwrapped) coordinates, making |d| huge -> tent = 0.

Layout: shard 512 rows/core (+1 halo row each side). On device, partitions =
128 column strips of 34 (32 owned + 1 halo col each side); free dim =
(rows, 34). All 9 stencil shifts are then free-dim offsets. ScalarE computes
the 6 tent weights, VectorE the 45 products in bf16 (2x mode, fused 3 taps
per instruction via stacked tiles + broadcast APs), TensorE accumulates the
9 taps per field into PSUM via identity matmuls; evac multiplies by mask*VP.

This toolchain's walrus allows ONE sync-wait per instruction; the
split_multi_waits pass hoists extra waits onto same-engine NOPs.
"""

import numpy as np
import ml_dtypes

from concourse import bass, mybir
from concourse import tile as tile_mod
from concourse.bass_utils import run_bass_kernel_spmd

NY, NX = 4096, 4096
VP = 3.1415 / 6.0
NCORES = 8
ROWS = NY // NCORES          # 512 output rows per core
P = 128                      # partitions = column strips
OW = NX // P                 # 32 output cols per strip
SW = OW + 2                  # 34 strip cols incl halo
SRC_ROWS = ROWS + 2          # 514 source rows per core

RCH = 64                     # output rows per chunk
SR = RCH + 2                 # source rows per chunk window
NCHUNK = ROWS // RCH
WIN = 16                     # psum window rows (16*32 = 512 = one psum bank)
NWIN = RCH // WIN

F32 = mybir.dt.float32
PROD_DT = mybir.dt.bfloat16  # dtype for weights/products
PROD_NP = ml_dtypes.bfloat16

_CACHE = {}


def split_multi_waits(nc):
    """Walrus allows one sync-wait per instruction; hoist extras onto NOPs."""
    for f in nc.m.functions:
        for bb in f.blocks:
            new = []
            for inst in bb.instructions:
                si = inst.sync_info
                if si is not None and si.on_wait and len(si.on_wait) > 1:
                    waits = list(si.on_wait)
                    for w in waits[:-1]:
                        nop = mybir.InstNoOp(name=f"W-{nc.next_id()}",
                                             ins=[], outs=[])
                        nop.engine = inst.engine
                        nop.sync_info = mybir.SyncInfo(on_wait=[w],
                                                       on_update=[])
                        new.append(nop)
                    inst.sync_info = mybir.SyncInfo(
                        on_wait=[waits[-1]], on_update=list(si.on_update))
                new.append(inst)
            bb.instructions = new


def _build_graph(repeat=1):
    nc = bass.Bass()
    halo_shape = [P, SRC_ROWS, SW]
    ux_d = nc.declare_dram_parameter("ux", halo_shape, F32, isOutput=False)
    uy_d = nc.declare_dram_parameter("uy", halo_shape, F32, isOutput=False)
    v_d = [nc.declare_dram_parameter(n, halo_shape, F32, isOutput=False)
           for n in ("vx", "vy", "fx", "fy")]
    mask_d = nc.declare_dram_parameter("mask", [P, ROWS, OW], F32, isOutput=False)
    eye_d = nc.declare_dram_parameter("eye", [P, P], PROD_DT, isOutput=False)
    out_d = nc.declare_dram_parameter("out", [5, P, ROWS, OW], PROD_DT,
                                      isOutput=True)

    # taps ordered sy-major so W9[:, 3*i + j] = R(sy_i) * C(sx_j)
    SYS = (1, 0, -1)
    SXS = (1, 0, -1)
    TAPS = [(sy, sx) for sy in SYS for sx in SXS]

    with tile_mod.TileContext(nc) as tc:
        with (
            tc.tile_pool(name="const", bufs=1) as cpool,
            tc.tile_pool(name="ldxy", bufs=2) as ldxy,
            tc.tile_pool(name="ldv", bufs=2) as ldv,
            tc.tile_pool(name="ldm", bufs=1) as ldm,
            tc.tile_pool(name="wk", bufs=1) as wkpool,
            tc.tile_pool(name="wt", bufs=1) as wtpool,
            tc.tile_pool(name="prod", bufs=3) as ppool,
            tc.tile_pool(name="outp", bufs=4) as opool,
            tc.tile_pool(name="psum", bufs=6, space="PSUM") as pspool,
        ):
            eye_t = cpool.tile([P, P], PROD_DT, tag="eye")
            nc.sync.dma_start(eye_t[:], eye_d[:])
            # per-partition-scalar bias columns (-s for s in 1,0,-1), ACT-owned
            bias_g = cpool.tile([P, 3], F32, tag="bias_g")
            for k, s in enumerate(SXS):
                nc.gpsimd.memset(bias_g[:, k:k + 1], float(-s))
            bias_t = cpool.tile([P, 3], F32, tag="bias")
            nc.scalar.copy(bias_t[:], bias_g[:])
            bias_col = {s: bias_t[:, k:k + 1] for k, s in enumerate(SXS)}

            for t in range(NCHUNK * repeat):
                t = t % NCHUNK
                r0 = t * RCH
                ux_t = ldxy.tile([P, SR, SW], PROD_DT, tag="ux")
                uy_t = ldxy.tile([P, SR, SW], PROD_DT, tag="uy")
                nc.gpsimd.dma_start(ux_t[:], ux_d[:, r0:r0 + SR, :])
                nc.gpsimd.dma_start(uy_t[:], uy_d[:, r0:r0 + SR, :])
                v_t = []
                for k, vd in enumerate(v_d):
                    vt = ldv.tile([P, 1, SR, SW], PROD_DT, tag=f"v{k}")
                    nc.gpsimd.dma_start(vt[:, 0], vd[:, r0:r0 + SR, :])
                    v_t.append(vt)
                mask_t = ldm.tile([P, RCH, OW], F32, tag="mask")
                nc.sync.dma_start(mask_t[:], mask_d[:, r0:r0 + RCH, :])
                # mc = mask * VP on ACT (keeps DVE off 2-port modes)
                mc_t = wkpool.tile([P, RCH, OW], PROD_DT, tag="mc")
                nc.scalar.mul(mc_t[:], mask_t[:], float(VP))

                # tent weights on ScalarE: tent(d) = relu(1 - |d|), stacked
                CW = wtpool.tile([P, 3, SR, SW], PROD_DT, tag="CW")
                RW = wtpool.tile([P, 3, SR, SW], PROD_DT, tag="RW")
                for k, s in enumerate(SXS):
                    ax = wkpool.tile([P, SR, SW], PROD_DT, tag="scr")
                    nc.scalar.activation(ax[:], ux_t[:],
                                         mybir.ActivationFunctionType.Abs,
                                         bias=bias_col[s])
                    nc.scalar.activation(CW[:, k], ax[:],
                                         mybir.ActivationFunctionType.Relu,
                                         bias=1.0, scale=-1.0)
                    ay = wkpool.tile([P, SR, SW], PROD_DT, tag="scr")
                    nc.scalar.activation(ay[:], uy_t[:],
                                         mybir.ActivationFunctionType.Abs,
                                         bias=bias_col[s])
                    nc.scalar.activation(RW[:, k], ay[:],
                                         mybir.ActivationFunctionType.Relu,
                                         bias=1.0, scale=-1.0)

                # 9 tap weights W9[:, 3i+j] = RW[i] * CW[j], 3 fused muls
                W9 = wtpool.tile([P, 9, SR, SW], PROD_DT, tag="W9")
                for i in range(3):
                    nc.vector.tensor_mul(
                        W9[:, 3 * i:3 * i + 3],
                        RW[:, i:i + 1].to_broadcast((P, 3, SR, SW)),
                        CW[:])

                for f in range(5):
                    ps = []
                    for w in range(NWIN):
                        ps_w = pspool.tile([P, WIN, OW], F32, tag="ps")
                        ps.append(ps_w)
                    for g in range(3):          # tap group = fixed sy
                        if f == 0:
                            src = W9[:, 3 * g:3 * g + 3]
                        else:
                            t3 = ppool.tile([P, 3, SR, SW], PROD_DT, tag="t3")
                            nc.vector.tensor_mul(
                                t3[:], W9[:, 3 * g:3 * g + 3],
                                v_t[f - 1][:].to_broadcast((P, 3, SR, SW)))
                            src = t3[:]
                        d = 1 - SYS[g]
                        for j in range(3):
                            ti = 3 * g + j
                            c0 = 1 - SXS[j]
                            for w in range(NWIN):
                                nc.tensor.matmul(
                                    ps[w][:],
                                    eye_t[:],
                                    src[:, j, w * WIN + d: w * WIN + d + WIN,
                                        c0:c0 + OW],
                                    start=(ti == 0), stop=(ti == 8),
                                )
                    for w in range(NWIN):
                        # ACT evacuates PSUM to bf16 SBUF; DVE multiplies in
                        # all-bf16 2x mode; output stored bf16, upcast on host
                        sv = opool.tile([P, WIN, OW], PROD_DT, tag="sv")
                        nc.scalar.copy(sv[:], ps[w][:])
                        ot = opool.tile([P, WIN, OW], PROD_DT, tag="out")
                        nc.vector.tensor_mul(
                            ot[:], sv[:],
                            mc_t[:, w * WIN:(w + 1) * WIN, :])
                        nc.sync.dma_start(
                            out_d[f, :, r0 + w * WIN:r0 + (w + 1) * WIN, :],
                            ot[:])
    split_multi_waits(nc)
    return nc


def _strips_halo(arr2d):
    """(rows, NX) f32 -> [P, rows, SW] with 1-col circular halos."""
    rows = arr2d.shape[0]
    pad = np.empty((rows, NX + 2), np.float32)
    pad[:, 1:NX + 1] = arr2d
    pad[:, 0] = arr2d[:, NX - 1]
    pad[:, NX + 1] = arr2d[:, 0]
    st = pad.strides
    v = np.lib.stride_tricks.as_strided(
        pad, shape=(P, rows, SW), strides=(OW * st[1], st[0], st[1]))
    return np.ascontiguousarray(v)


# virtual (unwrapped) column coordinate of each strip cell: 32p - 1 + c
_XC = (OW * np.arange(P, dtype=np.float32)[:, None, None] - 1.0
       + np.arange(SW, dtype=np.float32)[None, None, :])


def build_in_maps(xp_grid, yp_grid, vx_grid, vy_grid, Fx_grid, Fy_grid, mask):
    xp = np.asarray(xp_grid, np.float32).reshape(NY, NX)
    yp = np.asarray(yp_grid, np.float32).reshape(NY, NX)
    fields = [np.asarray(a, np.float32).reshape(NY, NX)
              for a in (vx_grid, vy_grid, Fx_grid, Fy_grid)]
    mk = np.asarray(mask, np.float32).reshape(NY, NX)
    eye = np.eye(P, dtype=PROD_NP)

    in_maps = []
    for i in range(NCORES):
        r0 = i * ROWS
        ridx = np.arange(r0 - 1, r0 + ROWS + 1) % NY
        ux = _strips_halo(xp[ridx])
        ux -= _XC                      # xp - x (virtual, unwrapped)
        uy = _strips_halo(yp[ridx])
        uy -= (np.arange(r0 - 1, r0 + ROWS + 1, dtype=np.float32)
               [None, :, None])       # yp - y (virtual, unwrapped)
        m = {
            "ux": ux,
            "uy": uy,
            "mask": np.ascontiguousarray(
                mk[r0:r0 + ROWS].reshape(ROWS, P, OW).transpose(1, 0, 2)),
            "eye": eye,
        }
        for nm, f in zip(("vx", "vy", "fx", "fy"), fields):
            m[nm] = _strips_halo(f[ridx])
        in_maps.append(m)
    return in_maps


def kernel(xp_grid, yp_grid, x_grid, y_grid, vx_grid, vy_grid, Fx_grid,
           Fy_grid, mask):
    if "nc" not in _CACHE:
        _CACHE["nc"] = _build_graph()
    nc = _CACHE["nc"]
    in_maps = build_in_maps(xp_grid, yp_grid, vx_grid, vy_grid, Fx_grid,
                            Fy_grid, mask)
    res = run_bass_kernel_spmd(nc, in_maps, core_ids=list(range(NCORES)))
    out = np.empty((5, 1, 1, NY, NX), np.float32)
    for i in range(NCORES):
        o = np.asarray(res.results[i]["out"]).astype(np.float32)
        out[:, 0, 0, i * ROWS:(i + 1) * ROWS, :] = (
            o.transpose(0, 2, 1, 3).reshape(5, ROWS, NX))
    return out


# revision 20
# speedup vs baseline: 5.6424x; 5.2178x over previous
"""AI4DEM coupling_backward on 8 TRN2 NeuronCores.

Math: for each of 9 taps (sy,sx) in {-1,0,1}^2, the reference computes
  w = N(roll(xp)-x) * N(roll(yp)-y) * mask * VP   with N(d) = max(0, 1-|d|)
  out = [sum w, sum w*roll(vx), sum w*roll(vy), sum w*roll(Fx), sum w*roll(Fy)]
Per-source factorization with ux = xp - x, uy = yp - y (computed during
host-side sharding, identical f32 op to the reference's roll(xp)-x):
  C_sx = tent(ux - sx), R_sy = tent(uy - sy)
  out_f[y,x] = mask[y,x]*VP * sum_{sy,sx} (C_sx*R_sy*v_f)[y-sy, x-sx]
Circular wrap taps vanish automatically because halo cells carry *virtual*
(unwrapped) coordinates, making |d| huge -> tent = 0.

Layout: shard 512 rows/core (+1 halo row each side). On device, partitions =
128 column strips of 34 (32 owned + 1 halo col each side); free dim =
(rows, 34). All 9 stencil shifts are then free-dim offsets. ScalarE computes
the 6 tent weights, VectorE the 45 products in bf16 (2x mode, fused 3 taps
per instruction via stacked tiles + broadcast APs), TensorE accumulates the
9 taps per field into PSUM via identity matmuls; evac multiplies by mask*VP.

This toolchain's walrus allows ONE sync-wait per instruction; the
split_multi_waits pass hoists extra waits onto same-engine NOPs.
"""

import numpy as np
import ml_dtypes

from concourse import bass, mybir
from concourse import tile as tile_mod
from concourse.bass_utils import run_bass_kernel_spmd

NY, NX = 4096, 4096
VP = 3.1415 / 6.0
NCORES = 8
ROWS = NY // NCORES          # 512 output rows per core
P = 128                      # partitions = column strips
OW = NX // P                 # 32 output cols per strip
SW = OW + 2                  # 34 strip cols incl halo
SRC_ROWS = ROWS + 2          # 514 source rows per core

RCH = 64                     # output rows per chunk
SR = RCH + 2                 # source rows per chunk window
NCHUNK = ROWS // RCH
WIN = 16                     # psum window rows (16*32 = 512 = one psum bank)
NWIN = RCH // WIN

F32 = mybir.dt.float32
PROD_DT = mybir.dt.bfloat16  # dtype for weights/products
PROD_NP = ml_dtypes.bfloat16

_CACHE = {}


def split_multi_waits(nc):
    """Walrus allows one sync-wait per instruction; hoist extras onto NOPs."""
    for f in nc.m.functions:
        for bb in f.blocks:
            new = []
            for inst in bb.instructions:
                si = inst.sync_info
                if si is not None and si.on_wait and len(si.on_wait) > 1:
                    waits = list(si.on_wait)
                    for w in waits[:-1]:
                        nop = mybir.InstNoOp(name=f"W-{nc.next_id()}",
                                             ins=[], outs=[])
                        nop.engine = inst.engine
                        nop.sync_info = mybir.SyncInfo(on_wait=[w],
                                                       on_update=[])
                        new.append(nop)
                    inst.sync_info = mybir.SyncInfo(
                        on_wait=[waits[-1]], on_update=list(si.on_update))
                new.append(inst)
            bb.instructions = new


def _build_graph(repeat=1):
    nc = bass.Bass()
    halo_shape = [P, SRC_ROWS, SW]
    ux_d = nc.declare_dram_parameter("ux", halo_shape, PROD_DT, isOutput=False)
    uy_d = nc.declare_dram_parameter("uy", halo_shape, PROD_DT, isOutput=False)
    v_d = [nc.declare_dram_parameter(n, halo_shape, PROD_DT, isOutput=False)
           for n in ("vx", "vy", "fx", "fy")]
    mask_d = nc.declare_dram_parameter("mask", [P, ROWS, OW], PROD_DT,
                                       isOutput=False)
    eye_d = nc.declare_dram_parameter("eye", [P, P], PROD_DT, isOutput=False)
    out_d = nc.declare_dram_parameter("out", [5, P, ROWS, OW], PROD_DT,
                                      isOutput=True)

    # taps ordered sy-major so W9[:, 3*i + j] = R(sy_i) * C(sx_j)
    SYS = (1, 0, -1)
    SXS = (1, 0, -1)
    TAPS = [(sy, sx) for sy in SYS for sx in SXS]

    with tile_mod.TileContext(nc) as tc:
        with (
            tc.tile_pool(name="const", bufs=1) as cpool,
            tc.tile_pool(name="ldxy", bufs=2) as ldxy,
            tc.tile_pool(name="ldv", bufs=2) as ldv,
            tc.tile_pool(name="ldm", bufs=1) as ldm,
            tc.tile_pool(name="wk", bufs=1) as wkpool,
            tc.tile_pool(name="wt", bufs=1) as wtpool,
            tc.tile_pool(name="prod", bufs=3) as ppool,
            tc.tile_pool(name="outp", bufs=4) as opool,
            tc.tile_pool(name="psum", bufs=6, space="PSUM") as pspool,
        ):
            eye_t = cpool.tile([P, P], PROD_DT, tag="eye")
            nc.sync.dma_start(eye_t[:], eye_d[:])
            # per-partition-scalar bias columns (-s for s in 1,0,-1), ACT-owned
            bias_g = cpool.tile([P, 3], F32, tag="bias_g")
            for k, s in enumerate(SXS):
                nc.gpsimd.memset(bias_g[:, k:k + 1], float(-s))
            bias_t = cpool.tile([P, 3], F32, tag="bias")
            nc.scalar.copy(bias_t[:], bias_g[:])
            bias_col = {s: bias_t[:, k:k + 1] for k, s in enumerate(SXS)}

            for t in range(NCHUNK * repeat):
                t = t % NCHUNK
                r0 = t * RCH
                ux_t = ldxy.tile([P, SR, SW], PROD_DT, tag="ux")
                uy_t = ldxy.tile([P, SR, SW], PROD_DT, tag="uy")
                nc.sync.dma_start(ux_t[:], ux_d[:, r0:r0 + SR, :])
                nc.sync.dma_start(uy_t[:], uy_d[:, r0:r0 + SR, :])
                v_t = []
                for k, vd in enumerate(v_d):
                    vt = ldv.tile([P, 1, SR, SW], PROD_DT, tag=f"v{k}")
                    nc.sync.dma_start(vt[:, 0], vd[:, r0:r0 + SR, :])
                    v_t.append(vt)
                mask_t = ldm.tile([P, RCH, OW], PROD_DT, tag="mask")
                nc.sync.dma_start(mask_t[:], mask_d[:, r0:r0 + RCH, :])
                # mc = mask * VP on ACT (keeps DVE off 2-port modes)
                mc_t = wkpool.tile([P, RCH, OW], PROD_DT, tag="mc")
                nc.scalar.mul(mc_t[:], mask_t[:], float(VP))

                # tent weights on ScalarE: tent(d) = relu(1 - |d|), stacked
                CW = wtpool.tile([P, 3, SR, SW], PROD_DT, tag="CW")
                RW = wtpool.tile([P, 3, SR, SW], PROD_DT, tag="RW")
                for k, s in enumerate(SXS):
                    ax = wkpool.tile([P, SR, SW], PROD_DT, tag="scr")
                    nc.scalar.activation(ax[:], ux_t[:],
                                         mybir.ActivationFunctionType.Abs,
                                         bias=bias_col[s])
                    nc.scalar.activation(CW[:, k], ax[:],
                                         mybir.ActivationFunctionType.Relu,
                                         bias=1.0, scale=-1.0)
                    ay = wkpool.tile([P, SR, SW], PROD_DT, tag="scr")
                    nc.scalar.activation(ay[:], uy_t[:],
                                         mybir.ActivationFunctionType.Abs,
                                         bias=bias_col[s])
                    nc.scalar.activation(RW[:, k], ay[:],
                                         mybir.ActivationFunctionType.Relu,
                                         bias=1.0, scale=-1.0)

                # 9 tap weights W9[:, 3i+j] = RW[i] * CW[j], 3 fused muls
                W9 = wtpool.tile([P, 9, SR, SW], PROD_DT, tag="W9")
                for i in range(3):
                    nc.vector.tensor_mul(
                        W9[:, 3 * i:3 * i + 3],
                        RW[:, i:i + 1].to_broadcast((P, 3, SR, SW)),
                        CW[:])

                for f in range(5):
                    ps = []
                    for w in range(NWIN):
                        ps_w = pspool.tile([P, WIN, OW], F32, tag="ps")
                        ps.append(ps_w)
                    for g in range(3):          # tap group = fixed sy
                        if f == 0:
                            src = W9[:, 3 * g:3 * g + 3]
                        else:
                            t3 = ppool.tile([P, 3, SR, SW], PROD_DT, tag="t3")
                            nc.vector.tensor_mul(
                                t3[:], W9[:, 3 * g:3 * g + 3],
                                v_t[f - 1][:].to_broadcast((P, 3, SR, SW)))
                            src = t3[:]
                        d = 1 - SYS[g]
                        for j in range(3):
                            ti = 3 * g + j
                            c0 = 1 - SXS[j]
                            for w in range(NWIN):
                                nc.tensor.matmul(
                                    ps[w][:],
                                    eye_t[:],
                                    src[:, j, w * WIN + d: w * WIN + d + WIN,
                                        c0:c0 + OW],
                                    start=(ti == 0), stop=(ti == 8),
                                )
                    for w in range(NWIN):
                        # ACT evacuates PSUM to bf16 SBUF; DVE multiplies in
                        # all-bf16 2x mode; output stored bf16, upcast on host
                        sv = opool.tile([P, WIN, OW], PROD_DT, tag="sv")
                        nc.scalar.copy(sv[:], ps[w][:])
                        ot = opool.tile([P, WIN, OW], PROD_DT, tag="out")
                        nc.vector.tensor_mul(
                            ot[:], sv[:],
                            mc_t[:, w * WIN:(w + 1) * WIN, :])
                        nc.sync.dma_start(
                            out_d[f, :, r0 + w * WIN:r0 + (w + 1) * WIN, :],
                            ot[:])
    split_multi_waits(nc)
    return nc


def _strips_halo(arr2d):
    """(rows, NX) f32 -> [P, rows, SW] with 1-col circular halos."""
    rows = arr2d.shape[0]
    pad = np.empty((rows, NX + 2), np.float32)
    pad[:, 1:NX + 1] = arr2d
    pad[:, 0] = arr2d[:, NX - 1]
    pad[:, NX + 1] = arr2d[:, 0]
    st = pad.strides
    v = np.lib.stride_tricks.as_strided(
        pad, shape=(P, rows, SW), strides=(OW * st[1], st[0], st[1]))
    return np.ascontiguousarray(v)


# virtual (unwrapped) column coordinate of each strip cell: 32p - 1 + c
_XC = (OW * np.arange(P, dtype=np.float32)[:, None, None] - 1.0
       + np.arange(SW, dtype=np.float32)[None, None, :])


def build_in_maps(xp_grid, yp_grid, vx_grid, vy_grid, Fx_grid, Fy_grid, mask):
    xp = np.asarray(xp_grid, np.float32).reshape(NY, NX)
    yp = np.asarray(yp_grid, np.float32).reshape(NY, NX)
    fields = [np.asarray(a, np.float32).reshape(NY, NX)
              for a in (vx_grid, vy_grid, Fx_grid, Fy_grid)]
    mk = np.asarray(mask, np.float32).reshape(NY, NX)
    eye = np.eye(P, dtype=PROD_NP)

    in_maps = []
    for i in range(NCORES):
        r0 = i * ROWS
        ridx = np.arange(r0 - 1, r0 + ROWS + 1) % NY
        ux = _strips_halo(xp[ridx])
        ux -= _XC                      # xp - x (virtual, unwrapped)
        uy = _strips_halo(yp[ridx])
        uy -= (np.arange(r0 - 1, r0 + ROWS + 1, dtype=np.float32)
               [None, :, None])       # yp - y (virtual, unwrapped)
        m = {
            "ux": ux.astype(PROD_NP),
            "uy": uy.astype(PROD_NP),
            "mask": mk[r0:r0 + ROWS].reshape(ROWS, P, OW)
                    .transpose(1, 0, 2).astype(PROD_NP),
            "eye": eye,
        }
        for nm, f in zip(("vx", "vy", "fx", "fy"), fields):
            m[nm] = _strips_halo(f[ridx]).astype(PROD_NP)
        in_maps.append(m)
    return in_maps


def kernel(xp_grid, yp_grid, x_grid, y_grid, vx_grid, vy_grid, Fx_grid,
           Fy_grid, mask):
    if "nc" not in _CACHE:
        _CACHE["nc"] = _build_graph()
    nc = _CACHE["nc"]
    in_maps = build_in_maps(xp_grid, yp_grid, vx_grid, vy_grid, Fx_grid,
                            Fy_grid, mask)
    res = run_bass_kernel_spmd(nc, in_maps, core_ids=list(range(NCORES)))
    out = np.empty((5, 1, 1, NY, NX), np.float32)
    for i in range(NCORES):
        o = np.asarray(res.results[i]["out"]).astype(np.float32)
        out[:, 0, 0, i * ROWS:(i + 1) * ROWS, :] = (
            o.transpose(0, 2, 1, 3).reshape(5, ROWS, NX))
    return out
